# revision 19
# baseline (speedup 1.0000x reference)
"""Trainium2 Bass kernel for nn_DynamicHippocampus (spiking hippocampus network).

Network: EC --pp--> DG --mf--> CA3 (--rc--> CA3) --sc--> CA1, T=4 Izhikevich
steps, output = final CA1 membrane potential.

Strategy
--------
The only data-dependent, non-uniform computation in this network is the EC
population (per-neuron drive).  DG/CA3/CA1 all start from identical state
(v=-65, u=-13) and receive spatially-uniform input for as long as no source
population has spiked (their synaptic currents are exactly zero, and the
inhibitory LIF populations provably stay at zero as well).  So while no spikes
have occurred, DG/CA3/CA1 evolve as uniform "scalar" populations.

The device kernel (SPMD over 8 NeuronCores, EC sharded by neuron index)
computes:
  * the full per-neuron EC Izhikevich dynamics for its EC shard, and the
    per-step EC spike counts (the certificate that the no-spike regime holds),
  * the uniform DG/CA3/CA1 scalar chains (replicated, ~3 lanes of work),
  * its shard of the CA1 output (broadcast of the uniform CA1 potential).

The host verifies the certificate (device-reported EC spike counts are zero
for every step, and the uniform chains never cross threshold).  If the
certificate holds -- it does for any drive bounded well below ~190, and the
model's drive is < 20 -- the device output is exact.  Otherwise kernel()
falls back to a bit-exact reference simulation on host (slow path; never
taken for in-distribution inputs, kept for correctness on arbitrary ones).
"""

import numpy as np

# population sizes (must match the model)
N_EC, N_DG, N_CA3, N_CA1 = 100000, 400000, 120000, 100000
N_I_DG, N_I_CA3, N_I_CA1 = 10000, 3000, 2000
T, DT = 4, 0.5
A, B = 0.02, 0.2
TAU_I, THR_I, INH_GAIN = 0.9, 1.0, 2.0

N_CORES = 8
P = 128          # SBUF partitions
COLS = 98        # free-dim columns of EC neurons per core
CH = 3           # extra columns carrying the uniform DG/CA3/CA1 chains
COLS_ALL = COLS + CH
EC_CORE = P * COLS            # 12544 EC neurons per core
EC_PAD = N_CORES * EC_CORE    # 100352 >= N_EC

_BUILT = None


def _build_program():
    """Build the (per-core identical) Bass program once.

    Single-engine (DVE) izh update over a [128, 101] state tile: 98 columns of
    EC neurons plus 3 columns carrying the uniform DG/CA3/CA1 chains (their
    drive columns are zero).  The host pre-scales drive to 0.5*I + 70 so the
    update fuses into 9 DVE ops per step:
        v' = clip(v*v*0.02 + 3.5*v + (0.5*I + 70 - 0.5*u), -90, 40)
        u' = 0.99*u + 0.002*v'
    """
    import contextlib

    import concourse.bass as bass
    import concourse.mybir as mybir

    f32 = mybir.dt.float32
    Alu = mybir.AluOpType
    X = mybir.AxisListType.X

    # The kernel's cross-engine dependencies are fully semaphore-protected
    # (DMA-in -> DVE -> DMA-out), so the framework's all-engine barriers
    # (const-AP init, block entry/exit) only serialize engine boot; skip them.
    class _NoBarrierBass(bass.Bass):
        def all_engine_barrier(self, *, sem_only: bool = False):
            pass

    nc = _NoBarrierBass(detect_race_conditions=False)
    drive_d = nc.declare_dram_parameter("drive", [P, T * COLS_ALL], f32, isOutput=False)
    cnt_d = nc.declare_dram_parameter("cnt_out", [P, T], f32, isOutput=True)
    c1_d = nc.declare_dram_parameter("c1_out", [P, COLS], f32, isOutput=True)

    stk = contextlib.ExitStack()
    with stk:
        drv = stk.enter_context(nc.sbuf_tensor([P, T * COLS_ALL], f32))
        v = stk.enter_context(nc.sbuf_tensor([P, COLS_ALL], f32))
        uu = stk.enter_context(nc.sbuf_tensor([P, COLS_ALL], f32))
        cnt = stk.enter_context(nc.sbuf_tensor([P, T], f32))
        sq = stk.enter_context(nc.sbuf_tensor([P, COLS_ALL], f32))
        w = stk.enter_context(nc.sbuf_tensor([P, COLS_ALL], f32))
        msk = stk.enter_context(nc.sbuf_tensor([P, COLS_ALL], f32))
        c1t = stk.enter_context(nc.sbuf_tensor([P, COLS], f32))
        dma_in = stk.enter_context(nc.semaphore("dma_in"))
        dve_done = stk.enter_context(nc.semaphore("dve_done"))
        dma_out = stk.enter_context(nc.semaphore("dma_out"))
        block = stk.enter_context(nc.Block(no_gpsimd_drain=True))

        @block.sync
        def _(sync):
            # split the drive load so step-0 compute overlaps the bulk DMA
            sync.dma_start(drv[:, :COLS_ALL], drive_d[:, :COLS_ALL]).then_inc(dma_in, 16)
            sync.dma_start(drv[:, COLS_ALL:], drive_d[:, COLS_ALL:]).then_inc(dma_in, 16)
            sync.wait_ge(dve_done, 1)
            sync.dma_start(c1_d[:], c1t[:]).then_inc(dma_out, 16)
            sync.dma_start(cnt_d[:], cnt[:]).then_inc(dma_out, 16)
            sync.wait_ge(dma_out, 32)

        @block.vector
        def _(vector):
            # DVE drains its 8-stage pipe after every op (next op can't issue
            # until the flush completes), so same-engine RAW needs no
            # semaphores; only the DMA boundaries are synced.
            #
            # State:  v = membrane potential, uu = 500*u (scaled recovery so
            # its update fuses into one op: uu' = 0.99*uu + v').
            # Update: v' = clip(0.02 v^2 + 3.5 v + (0.5 I + 70) - 0.001 uu,
            #                   -90, 40)
            # Step 0 is closed-form: v=-65, u=-13 everywhere, so
            # v'0 = (0.5 I + 70) - 136.5 and uu'0 = v'0 - 6435.
            vector.wait_ge(dma_in, 16)
            for t in range(T):
                if t == 1:
                    vector.wait_ge(dma_in, 32)
                I_t = drv[:, t * COLS_ALL:(t + 1) * COLS_ALL]
                if t == 0:
                    vector.tensor_scalar(v[:], I_t, -136.5, None, Alu.add)
                else:
                    # w = -0.001*uu + (0.5*I + 70)    [host pre-scaled drive]
                    vector.scalar_tensor_tensor(w[:], uu[:], -0.001, I_t,
                                                Alu.mult, Alu.add)
                    # sq = v*v ; w = 0.02*sq + w ; v = 3.5*v + w
                    vector.tensor_tensor(sq[:], v[:], v[:], op=Alu.mult)
                    vector.scalar_tensor_tensor(w[:], sq[:], 0.02, w[:],
                                                Alu.mult, Alu.add)
                    vector.scalar_tensor_tensor(v[:], v[:], 3.5, w[:],
                                                Alu.mult, Alu.add)
                # clip to [-90, 40]
                vector.tensor_scalar(v[:], v[:], 40.0, -90.0, Alu.min, Alu.max)
                # spike mask; accum_out gives the per-partition spike count
                vector.tensor_scalar(msk[:], v[:], 30.0, None, Alu.is_ge,
                                     Alu.add, accum_out=cnt[:, t:t + 1])
                # uu' = 0.99*uu + v'   (uu0 = 500*(-13) = -6500)
                if t == 0:
                    vector.tensor_scalar(uu[:], v[:], -6435.0, None, Alu.add)
                else:
                    vector.scalar_tensor_tensor(uu[:], uu[:], 0.99, v[:],
                                                Alu.mult, Alu.add)

            # CA1 output column; signals that cnt and c1t are ready
            vector.tensor_copy(
                c1t[:], v[:, COLS + 2:COLS + 3].to_broadcast((P, COLS))
            ).then_inc(dve_done, 1)

    return nc


def _get_program():
    global _BUILT
    if _BUILT is None:
        _BUILT = _build_program()
    return _BUILT


def _host_uniform_chain(max_abs_drive):
    """Replicates the uniform DG/CA3/CA1 dynamics in f32 on host.

    Returns (c1_v_scalar, clean) where clean additionally certifies that no
    uniform population or inhibitory LIF unit could have crossed threshold.
    """
    f = np.float32
    v = np.full(3, -65.0, f)
    u = np.full(3, -13.0, f)
    clean = True
    for _ in range(T):
        vp = np.clip(v + (f(0.04) * v * v + f(5.0) * v + f(140.0) - u) * f(DT),
                     -90.0, 40.0).astype(f)
        u = (u + f(A) * (f(B) * vp - u) * f(DT)).astype(f)
        if np.any(vp >= 29.0):  # margin below the 30.0 threshold
            clean = False
        v = vp
    # inhibitory LIF with zero input stays at 0 < THR_I; nothing to check.
    return float(v[2]), clean


def _reference_fallback(inputs):
    """Bit-faithful host replication of the reference model (slow path)."""
    f = np.float32
    d = inputs

    def transmit(spk, src, tgt, val, n_tgt):
        w = (val * spk[src]).astype(f)
        return np.bincount(tgt, weights=w, minlength=n_tgt).astype(f)

    def izh(v, u, c, dd, I):
        v = np.clip(v + (f(0.04) * v * v + f(5.0) * v + f(140.0) - u + I) * f(DT),
                    -90.0, 40.0).astype(f)
        u = (u + f(A) * (f(B) * v - u) * f(DT)).astype(f)
        s = (v >= 30.0).astype(f)
        return np.where(s > 0, c, v).astype(f), np.where(s > 0, u + dd, u).astype(f), s

    def lif(v, inp):
        v = (f(TAU_I) * v + f(1.0 - TAU_I) * inp).astype(f)
        s = (v >= THR_I).astype(f)
        return np.where(s > 0, 0.0, v).astype(f), s

    ec_v = np.full(N_EC, -65.0, f); ec_u = np.full(N_EC, B * -65.0, f)
    dg_v = np.full(N_DG, -65.0, f); dg_u = np.full(N_DG, B * -65.0, f)
    c3_v = np.full(N_CA3, -65.0, f); c3_u = np.full(N_CA3, B * -65.0, f)
    c1_v = np.full(N_CA1, -65.0, f); c1_u = np.full(N_CA1, B * -65.0, f)
    c3_s = np.zeros(N_CA3, f)
    iv_dg = np.zeros(N_I_DG, f); iv_c3 = np.zeros(N_I_CA3, f); iv_c1 = np.zeros(N_I_CA1, f)

    for t in range(T):
        I_ec = d["drive"][t]
        ec_v, ec_u, ec_s = izh(ec_v, ec_u, d["ec_c"], d["ec_d"], I_ec)
        dg_I = transmit(ec_s, d["pp_src"], d["pp_tgt"], d["pp_val"], N_DG)
        iv_dg, is_dg = lif(iv_dg, np.full(N_I_DG, dg_I.mean(), f))
        dg_v, dg_u, dg_s = izh(dg_v, dg_u, d["dg_c"], d["dg_d"],
                               dg_I - f(INH_GAIN) * is_dg.mean(dtype=f))
        c3_I = (transmit(dg_s, d["mf_src"], d["mf_tgt"], d["mf_val"], N_CA3)
                + transmit(c3_s, d["rc_src"], d["rc_tgt"], d["rc_val"], N_CA3))
        iv_c3, is_c3 = lif(iv_c3, np.full(N_I_CA3, c3_I.mean(), f))
        c3_v, c3_u, c3_s = izh(c3_v, c3_u, d["ca3_c"], d["ca3_d"],
                               c3_I - f(INH_GAIN) * is_c3.mean(dtype=f))
        c1_I = transmit(c3_s, d["sc_src"], d["sc_tgt"], d["sc_val"], N_CA1)
        iv_c1, is_c1 = lif(iv_c1, np.full(N_I_CA1, c1_I.mean(), f))
        c1_v, c1_u, c1_s = izh(c1_v, c1_u, d["ca1_c"], d["ca1_d"],
                               c1_I - f(INH_GAIN) * is_c1.mean(dtype=f))
    return c1_v


def make_in_maps(drive):
    """Per-core input maps: pre-scale drive to 0.5*I + 70 (the constant part
    of the izh update) and pad EC with silent neurons (I=0 -> prescaled 70,
    same init state -> never spike).  The 3 chain columns also carry I=0,
    i.e. 70 after prescale."""
    drive = np.asarray(drive, dtype=np.float32)
    drive_pre = (np.float32(0.5) * drive + np.float32(70.0)).astype(np.float32)
    drive_pad = np.full((T, EC_PAD), 70.0, np.float32)
    drive_pad[:, :N_EC] = drive_pre

    in_maps = []
    for k in range(N_CORES):
        shard = drive_pad[:, k * EC_CORE:(k + 1) * EC_CORE]      # [T, 12544]
        shard = shard.reshape(T, P, COLS).transpose(1, 0, 2)      # [P, T, COLS]
        arr = np.full((P, T, COLS_ALL), 70.0, np.float32)
        arr[:, :, :COLS] = shard
        in_maps.append({"drive": np.ascontiguousarray(arr).reshape(P, T * COLS_ALL)})
    return in_maps


def kernel(**inputs):
    from concourse.bass_utils import run_bass_kernel_spmd

    drive = np.asarray(inputs["drive"], dtype=np.float32)
    assert drive.shape == (T, N_EC)
    in_maps = make_in_maps(drive)
    nc = _get_program()
    res = run_bass_kernel_spmd(nc, in_maps, list(range(N_CORES)))

    counts = np.zeros(T, np.float64)
    c1_vals = []
    for k in range(N_CORES):
        out = res.results[k]
        counts += np.asarray(out["cnt_out"], np.float64).reshape(P, T).sum(axis=0)
        c1_vals.append(np.asarray(out["c1_out"], np.float32).reshape(-1))
    c1_vals = np.concatenate(c1_vals)  # uniform CA1 value, replicated per lane

    c1_scalar, chain_clean = _host_uniform_chain(float(np.abs(drive).max()))
    if counts.sum() == 0 and chain_clean and np.all(c1_vals == c1_vals[0]):
        # cross-check device uniform value against the host chain
        if abs(float(c1_vals[0]) - c1_scalar) > 1e-3:
            return _reference_fallback(inputs)
        return np.full(N_CA1, c1_vals[0], np.float32)
    # spikes occurred: exact (slow) host fallback
    return _reference_fallback(inputs)


# revision 20
# speedup vs baseline: 1.0618x; 1.0618x over previous
"""Trainium2 Bass kernel for nn_DynamicHippocampus (spiking hippocampus network).

Network: EC --pp--> DG --mf--> CA3 (--rc--> CA3) --sc--> CA1, T=4 Izhikevich
steps, output = final CA1 membrane potential.

Strategy
--------
The only data-dependent, non-uniform computation in this network is the EC
population (per-neuron drive).  DG/CA3/CA1 all start from identical state
(v=-65, u=-13) and receive spatially-uniform input for as long as no source
population has spiked (their synaptic currents are exactly zero, and the
inhibitory LIF populations provably stay at zero as well).  So while no spikes
have occurred, DG/CA3/CA1 evolve as uniform "scalar" populations.

The device kernel (SPMD over 8 NeuronCores, EC sharded by neuron index)
computes:
  * the full per-neuron EC Izhikevich dynamics for its EC shard, and the
    per-step EC spike counts (the certificate that the no-spike regime holds),
  * the uniform DG/CA3/CA1 scalar chains (replicated, ~3 lanes of work),
  * its shard of the CA1 output (broadcast of the uniform CA1 potential).

The host verifies the certificate (device-reported EC spike counts are zero
for every step, and the uniform chains never cross threshold).  If the
certificate holds -- it does for any drive bounded well below ~190, and the
model's drive is < 20 -- the device output is exact.  Otherwise kernel()
falls back to a bit-exact reference simulation on host (slow path; never
taken for in-distribution inputs, kept for correctness on arbitrary ones).
"""

import numpy as np

# population sizes (must match the model)
N_EC, N_DG, N_CA3, N_CA1 = 100000, 400000, 120000, 100000
N_I_DG, N_I_CA3, N_I_CA1 = 10000, 3000, 2000
T, DT = 4, 0.5
A, B = 0.02, 0.2
TAU_I, THR_I, INH_GAIN = 0.9, 1.0, 2.0

N_CORES = 8
P = 128          # SBUF partitions
COLS = 98        # free-dim columns of EC neurons per core
CH = 3           # extra columns carrying the uniform DG/CA3/CA1 chains
COLS_ALL = COLS + CH
EC_CORE = P * COLS            # 12544 EC neurons per core
EC_PAD = N_CORES * EC_CORE    # 100352 >= N_EC

_BUILT = None


def _build_program():
    """Build the (per-core identical) Bass program once.

    Single-engine (DVE) izh update over a [128, 101] state tile: 98 columns of
    EC neurons plus 3 columns carrying the uniform DG/CA3/CA1 chains (their
    drive columns are zero).  The host pre-scales drive to 0.5*I + 70 and the
    recovery variable is kept as uu = 500*u, so a step is ~7 fused DVE ops:
        v'  = clip(v*v*0.02 + 3.5*v + (0.5*I + 70) - 0.001*uu, -90, 40)
        uu' = 0.99*uu + v'
    (step 0 is closed-form since v,u start uniform).  Spike counts come from
    the compare op's accum_out; DMA-in is split so step-0 compute overlaps
    the bulk of the drive transfer.
    """
    import contextlib

    import concourse.bass as bass
    import concourse.mybir as mybir

    f32 = mybir.dt.float32
    Alu = mybir.AluOpType
    X = mybir.AxisListType.X

    # The kernel's cross-engine dependencies are fully semaphore-protected
    # (DMA-in -> DVE -> DMA-out), so the framework's all-engine barriers
    # (const-AP init, block entry/exit) only serialize engine boot; skip them.
    class _NoBarrierBass(bass.Bass):
        def all_engine_barrier(self, *, sem_only: bool = False):
            pass

    nc = _NoBarrierBass(detect_race_conditions=False)
    drive_d = nc.declare_dram_parameter("drive", [P, T * COLS_ALL], f32, isOutput=False)
    cnt_d = nc.declare_dram_parameter("cnt_out", [P, T], f32, isOutput=True)
    c1_d = nc.declare_dram_parameter("c1_out", [P, COLS], f32, isOutput=True)

    stk = contextlib.ExitStack()
    with stk:
        drv = stk.enter_context(nc.sbuf_tensor([P, T * COLS_ALL], f32))
        v = stk.enter_context(nc.sbuf_tensor([P, COLS_ALL], f32))
        uu = stk.enter_context(nc.sbuf_tensor([P, COLS_ALL], f32))
        cnt = stk.enter_context(nc.sbuf_tensor([P, T], f32))
        sq = stk.enter_context(nc.sbuf_tensor([P, COLS_ALL], f32))
        w = stk.enter_context(nc.sbuf_tensor([P, COLS_ALL], f32))
        msk = stk.enter_context(nc.sbuf_tensor([P, COLS_ALL], f32))
        c1t = stk.enter_context(nc.sbuf_tensor([P, COLS], f32))
        dma_in = stk.enter_context(nc.semaphore("dma_in"))
        dve_done = stk.enter_context(nc.semaphore("dve_done"))
        dma_out = stk.enter_context(nc.semaphore("dma_out"))
        block = stk.enter_context(nc.Block(no_gpsimd_drain=True))

        @block.sync
        def _(sync):
            # split the drive load so step-0 compute overlaps the bulk DMA
            sync.dma_start(drv[:, :COLS_ALL], drive_d[:, :COLS_ALL]).then_inc(dma_in, 16)
            sync.dma_start(drv[:, COLS_ALL:], drive_d[:, COLS_ALL:]).then_inc(dma_in, 16)
            sync.wait_ge(dve_done, 1)
            sync.dma_start(c1_d[:], c1t[:]).then_inc(dma_out, 16)
            sync.dma_start(cnt_d[:], cnt[:]).then_inc(dma_out, 16)
            sync.wait_ge(dma_out, 32)

        @block.vector
        def _(vector):
            # DVE drains its 8-stage pipe after every op (next op can't issue
            # until the flush completes), so same-engine RAW needs no
            # semaphores; only the DMA boundaries are synced.
            #
            # State:  v = membrane potential, uu = 500*u (scaled recovery so
            # its update fuses into one op: uu' = 0.99*uu + v').
            # Update: v' = clip(0.02 v^2 + 3.5 v + (0.5 I + 70) - 0.001 uu,
            #                   -90, 40)
            # Step 0 is closed-form: v=-65, u=-13 everywhere, so
            # v'0 = (0.5 I + 70) - 136.5 and uu'0 = v'0 - 6435.
            vector.wait_ge(dma_in, 16)
            for t in range(T):
                if t == 1:
                    vector.wait_ge(dma_in, 32)
                I_t = drv[:, t * COLS_ALL:(t + 1) * COLS_ALL]
                if t == 0:
                    vector.tensor_scalar(v[:], I_t, -136.5, None, Alu.add)
                else:
                    # w = -0.001*uu + (0.5*I + 70)    [host pre-scaled drive]
                    vector.scalar_tensor_tensor(w[:], uu[:], -0.001, I_t,
                                                Alu.mult, Alu.add)
                    # sq = v*v ; w = 0.02*sq + w ; v = 3.5*v + w
                    vector.tensor_tensor(sq[:], v[:], v[:], op=Alu.mult)
                    vector.scalar_tensor_tensor(w[:], sq[:], 0.02, w[:],
                                                Alu.mult, Alu.add)
                    vector.scalar_tensor_tensor(v[:], v[:], 3.5, w[:],
                                                Alu.mult, Alu.add)
                # clip to [-90, 40]
                vector.tensor_scalar(v[:], v[:], 40.0, -90.0, Alu.min, Alu.max)
                # spike mask; accum_out gives the per-partition spike count
                vector.tensor_scalar(msk[:], v[:], 30.0, None, Alu.is_ge,
                                     Alu.add, accum_out=cnt[:, t:t + 1])
                # uu' = 0.99*uu + v'   (uu0 = 500*(-13) = -6500)
                if t == 0:
                    vector.tensor_scalar(uu[:], v[:], -6435.0, None, Alu.add)
                else:
                    vector.scalar_tensor_tensor(uu[:], uu[:], 0.99, v[:],
                                                Alu.mult, Alu.add)

            # CA1 output column; signals that cnt and c1t are ready
            vector.tensor_copy(
                c1t[:], v[:, COLS + 2:COLS + 3].to_broadcast((P, COLS))
            ).then_inc(dve_done, 1)

    return nc


def _get_program():
    global _BUILT
    if _BUILT is None:
        _BUILT = _build_program()
    return _BUILT


def _host_uniform_chain(max_abs_drive):
    """Replicates the uniform DG/CA3/CA1 dynamics in f32 on host.

    Returns (c1_v_scalar, clean) where clean additionally certifies that no
    uniform population or inhibitory LIF unit could have crossed threshold.
    """
    f = np.float32
    v = np.full(3, -65.0, f)
    u = np.full(3, -13.0, f)
    clean = True
    for _ in range(T):
        vp = np.clip(v + (f(0.04) * v * v + f(5.0) * v + f(140.0) - u) * f(DT),
                     -90.0, 40.0).astype(f)
        u = (u + f(A) * (f(B) * vp - u) * f(DT)).astype(f)
        if np.any(vp >= 29.0):  # margin below the 30.0 threshold
            clean = False
        v = vp
    # inhibitory LIF with zero input stays at 0 < THR_I; nothing to check.
    return float(v[2]), clean


def _reference_fallback(inputs):
    """Bit-faithful host replication of the reference model (slow path)."""
    f = np.float32
    d = inputs

    def transmit(spk, src, tgt, val, n_tgt):
        w = (val * spk[src]).astype(f)
        return np.bincount(tgt, weights=w, minlength=n_tgt).astype(f)

    def izh(v, u, c, dd, I):
        v = np.clip(v + (f(0.04) * v * v + f(5.0) * v + f(140.0) - u + I) * f(DT),
                    -90.0, 40.0).astype(f)
        u = (u + f(A) * (f(B) * v - u) * f(DT)).astype(f)
        s = (v >= 30.0).astype(f)
        return np.where(s > 0, c, v).astype(f), np.where(s > 0, u + dd, u).astype(f), s

    def lif(v, inp):
        v = (f(TAU_I) * v + f(1.0 - TAU_I) * inp).astype(f)
        s = (v >= THR_I).astype(f)
        return np.where(s > 0, 0.0, v).astype(f), s

    ec_v = np.full(N_EC, -65.0, f); ec_u = np.full(N_EC, B * -65.0, f)
    dg_v = np.full(N_DG, -65.0, f); dg_u = np.full(N_DG, B * -65.0, f)
    c3_v = np.full(N_CA3, -65.0, f); c3_u = np.full(N_CA3, B * -65.0, f)
    c1_v = np.full(N_CA1, -65.0, f); c1_u = np.full(N_CA1, B * -65.0, f)
    c3_s = np.zeros(N_CA3, f)
    iv_dg = np.zeros(N_I_DG, f); iv_c3 = np.zeros(N_I_CA3, f); iv_c1 = np.zeros(N_I_CA1, f)

    for t in range(T):
        I_ec = d["drive"][t]
        ec_v, ec_u, ec_s = izh(ec_v, ec_u, d["ec_c"], d["ec_d"], I_ec)
        dg_I = transmit(ec_s, d["pp_src"], d["pp_tgt"], d["pp_val"], N_DG)
        iv_dg, is_dg = lif(iv_dg, np.full(N_I_DG, dg_I.mean(), f))
        dg_v, dg_u, dg_s = izh(dg_v, dg_u, d["dg_c"], d["dg_d"],
                               dg_I - f(INH_GAIN) * is_dg.mean(dtype=f))
        c3_I = (transmit(dg_s, d["mf_src"], d["mf_tgt"], d["mf_val"], N_CA3)
                + transmit(c3_s, d["rc_src"], d["rc_tgt"], d["rc_val"], N_CA3))
        iv_c3, is_c3 = lif(iv_c3, np.full(N_I_CA3, c3_I.mean(), f))
        c3_v, c3_u, c3_s = izh(c3_v, c3_u, d["ca3_c"], d["ca3_d"],
                               c3_I - f(INH_GAIN) * is_c3.mean(dtype=f))
        c1_I = transmit(c3_s, d["sc_src"], d["sc_tgt"], d["sc_val"], N_CA1)
        iv_c1, is_c1 = lif(iv_c1, np.full(N_I_CA1, c1_I.mean(), f))
        c1_v, c1_u, c1_s = izh(c1_v, c1_u, d["ca1_c"], d["ca1_d"],
                               c1_I - f(INH_GAIN) * is_c1.mean(dtype=f))
    return c1_v


def make_in_maps(drive):
    """Per-core input maps: pre-scale drive to 0.5*I + 70 (the constant part
    of the izh update) and pad EC with silent neurons (I=0 -> prescaled 70,
    same init state -> never spike).  The 3 chain columns also carry I=0,
    i.e. 70 after prescale."""
    drive = np.asarray(drive, dtype=np.float32)
    drive_pre = (np.float32(0.5) * drive + np.float32(70.0)).astype(np.float32)
    drive_pad = np.full((T, EC_PAD), 70.0, np.float32)
    drive_pad[:, :N_EC] = drive_pre

    in_maps = []
    for k in range(N_CORES):
        shard = drive_pad[:, k * EC_CORE:(k + 1) * EC_CORE]      # [T, 12544]
        shard = shard.reshape(T, P, COLS).transpose(1, 0, 2)      # [P, T, COLS]
        arr = np.full((P, T, COLS_ALL), 70.0, np.float32)
        arr[:, :, :COLS] = shard
        in_maps.append({"drive": np.ascontiguousarray(arr).reshape(P, T * COLS_ALL)})
    return in_maps


def kernel(**inputs):
    from concourse.bass_utils import run_bass_kernel_spmd

    drive = np.asarray(inputs["drive"], dtype=np.float32)
    assert drive.shape == (T, N_EC)
    in_maps = make_in_maps(drive)
    nc = _get_program()
    res = run_bass_kernel_spmd(nc, in_maps, list(range(N_CORES)))

    counts = np.zeros(T, np.float64)
    c1_vals = []
    for k in range(N_CORES):
        out = res.results[k]
        counts += np.asarray(out["cnt_out"], np.float64).reshape(P, T).sum(axis=0)
        c1_vals.append(np.asarray(out["c1_out"], np.float32).reshape(-1))
    c1_vals = np.concatenate(c1_vals)  # uniform CA1 value, replicated per lane

    c1_scalar, chain_clean = _host_uniform_chain(float(np.abs(drive).max()))
    if counts.sum() == 0 and chain_clean and np.all(c1_vals == c1_vals[0]):
        # cross-check device uniform value against the host chain
        if abs(float(c1_vals[0]) - c1_scalar) > 1e-3:
            return _reference_fallback(inputs)
        return np.full(N_CA1, c1_vals[0], np.float32)
    # spikes occurred: exact (slow) host fallback
    return _reference_fallback(inputs)


# revision 23
# speedup vs baseline: 1.0790x; 1.0162x over previous
"""Trainium2 Bass kernel for nn_DynamicHippocampus (spiking hippocampus network).

Network: EC --pp--> DG --mf--> CA3 (--rc--> CA3) --sc--> CA1, T=4 Izhikevich
steps, output = final CA1 membrane potential.

Strategy
--------
The only data-dependent, non-uniform computation in this network is the EC
population (per-neuron drive).  DG/CA3/CA1 all start from identical state
(v=-65, u=-13) and receive spatially-uniform input for as long as no source
population has spiked (their synaptic currents are exactly zero, and the
inhibitory LIF populations provably stay at zero as well).  So while no spikes
have occurred, DG/CA3/CA1 evolve as uniform "scalar" populations.

The device kernel (SPMD over 8 NeuronCores, EC sharded by neuron index)
computes:
  * the full per-neuron EC Izhikevich dynamics for its EC shard, and the
    per-step EC spike counts (the certificate that the no-spike regime holds),
  * the uniform DG/CA3/CA1 scalar chains (replicated, ~3 lanes of work),
  * its shard of the CA1 output (broadcast of the uniform CA1 potential).

The host verifies the certificate (device-reported EC spike counts are zero
for every step, and the uniform chains never cross threshold).  If the
certificate holds -- it does for any drive bounded well below ~190, and the
model's drive is < 20 -- the device output is exact.  Otherwise kernel()
falls back to a bit-exact reference simulation on host (slow path; never
taken for in-distribution inputs, kept for correctness on arbitrary ones).
"""

import numpy as np

# population sizes (must match the model)
N_EC, N_DG, N_CA3, N_CA1 = 100000, 400000, 120000, 100000
N_I_DG, N_I_CA3, N_I_CA1 = 10000, 3000, 2000
T, DT = 4, 0.5
A, B = 0.02, 0.2
TAU_I, THR_I, INH_GAIN = 0.9, 1.0, 2.0

# ACT computes sq2 = Square(a*v + b) = 0.02 v^2 + 3.5 v + b^2, so the DVE
# update is v' = sq2 + (0.5 I - (b^2 - 70)) - 0.001*uu.
_SQ_A = float(np.float32(np.sqrt(np.float32(0.02))))
_SQ_B = float(np.float32(3.5 / (2.0 * _SQ_A)))
_SQ_B2 = float(np.float32(_SQ_B) * np.float32(_SQ_B))
_IP0 = float(np.float32(70.0 - _SQ_B2))          # prescale constant for I=0

N_CORES = 8
P = 128          # SBUF partitions
COLS = 98        # free-dim columns of EC neurons per core
CH = 3           # extra columns carrying the uniform DG/CA3/CA1 chains
COLS_ALL = COLS + CH
EC_CORE = P * COLS            # 12544 EC neurons per core
EC_PAD = N_CORES * EC_CORE    # 100352 >= N_EC

_BUILT = None


def _build_program():
    """Build the (per-core identical) Bass program once.

    Single-engine (DVE) izh update over a [128, 101] state tile: 98 columns of
    EC neurons plus 3 columns carrying the uniform DG/CA3/CA1 chains (their
    drive columns are zero).  The host pre-scales drive to 0.5*I + 70 and the
    recovery variable is kept as uu = 500*u, so a step is ~7 fused DVE ops:
        v'  = clip(v*v*0.02 + 3.5*v + (0.5*I + 70) - 0.001*uu, -90, 40)
        uu' = 0.99*uu + v'
    (step 0 is closed-form since v,u start uniform).  Spike counts come from
    the compare op's accum_out; DMA-in is split so step-0 compute overlaps
    the bulk of the drive transfer.
    """
    import contextlib

    import concourse.bass as bass
    import concourse.mybir as mybir

    f32 = mybir.dt.float32
    Alu = mybir.AluOpType
    X = mybir.AxisListType.X

    # The kernel's cross-engine dependencies are fully semaphore-protected
    # (DMA-in -> DVE -> DMA-out), so the framework's all-engine barriers
    # (const-AP init, block entry/exit) only serialize engine boot; skip them.
    class _NoBarrierBass(bass.Bass):
        def all_engine_barrier(self, *, sem_only: bool = False):
            pass

    nc = _NoBarrierBass(detect_race_conditions=False)
    drive_d = nc.declare_dram_parameter("drive", [P, T * COLS_ALL], f32, isOutput=False)
    out_d = nc.declare_dram_parameter("out_all", [P, COLS + T], f32, isOutput=True)

    stk = contextlib.ExitStack()
    with stk:
        drv = stk.enter_context(nc.sbuf_tensor([P, T * COLS_ALL], f32))
        v = stk.enter_context(nc.sbuf_tensor([P, COLS_ALL], f32))
        uu = stk.enter_context(nc.sbuf_tensor([P, COLS_ALL], f32))
        cnt = stk.enter_context(nc.sbuf_tensor([P, T], f32))
        sq2 = stk.enter_context(nc.sbuf_tensor([P, COLS_ALL], f32))
        w = stk.enter_context(nc.sbuf_tensor([P, COLS_ALL], f32))
        msk = stk.enter_context(nc.sbuf_tensor([P, COLS_ALL], f32))
        outt = stk.enter_context(nc.sbuf_tensor([P, COLS + T], f32))
        act_a = stk.enter_context(nc.sbuf_tensor([P, 1], f32))
        act_b = stk.enter_context(nc.sbuf_tensor([P, 1], f32))
        dma_in = stk.enter_context(nc.semaphore("dma_in"))
        dve2act = stk.enter_context(nc.semaphore("dve2act"))
        act_sem = stk.enter_context(nc.semaphore("act_sem"))
        dve_done = stk.enter_context(nc.semaphore("dve_done"))
        dma_out = stk.enter_context(nc.semaphore("dma_out"))
        block = stk.enter_context(nc.Block(no_gpsimd_drain=True))

        Act = mybir.ActivationFunctionType

        @block.sync
        def _(sync):
            # 3-way split: step 0 / step 1 / steps 2-3, so compute overlaps DMA
            sync.dma_start(drv[:, :COLS_ALL],
                           drive_d[:, :COLS_ALL]).then_inc(dma_in, 16)
            sync.dma_start(drv[:, COLS_ALL:2 * COLS_ALL],
                           drive_d[:, COLS_ALL:2 * COLS_ALL]).then_inc(dma_in, 16)
            sync.dma_start(drv[:, 2 * COLS_ALL:],
                           drive_d[:, 2 * COLS_ALL:]).then_inc(dma_in, 16)
            sync.wait_ge(dve_done, 1)
            # no explicit completion wait: the NRT end-of-stream DRAIN on the
            # sync engine drains its DGE queues before the program retires
            sync.dma_start(out_d[:], outt[:]).then_inc(dma_out, 16)

        @block.scalar
        def _(scalar):
            # warm the Square LUT during the DMA window, then produce
            # sq2(t) = Square(a*v(t-1) + b) as soon as clip(t-1) lands.
            scalar.wait_ge(dve2act, 1)   # act_a/act_b written
            scalar.activation(sq2[:, 0:1], act_a[:], Act.Square,
                              bias=act_b[:], scale=act_a[:])
            for t in range(1, T):
                scalar.activation(
                    sq2[:], v[:], Act.Square, bias=act_b[:], scale=act_a[:]
                )._wait_ge(dve2act, t + 1).then_inc(act_sem, 1)

        @block.vector
        def _(vector):
            # DVE drains its 8-stage pipe after every op, so same-engine RAW
            # needs no semaphores; only DMA/ACT boundaries are synced.
            #
            # State:  v = membrane potential, uu = 500*u (scaled recovery so
            # its update fuses into one op: uu' = 0.99*uu + v').
            # Update: v' = clip(sq2 + Ip - 0.001*uu, -90, 40) with
            #         sq2 = 0.02 v^2 + 3.5 v + b^2 (ACT), Ip = 0.5*I + 70 - b^2.
            # Step 0 is closed-form: v=-65, u=-13 everywhere, so
            # v'0 = Ip + 16.625 and uu'0 = v'0 - 6435.
            vector.memset(act_a[:], _SQ_A)
            vector.memset(act_b[:], _SQ_B).then_inc(dve2act, 1)
            vector.wait_ge(dma_in, 16)
            for t in range(T):
                I_t = drv[:, t * COLS_ALL:(t + 1) * COLS_ALL]
                if t == 0:
                    vector.tensor_scalar(v[:], I_t, 16.625, None, Alu.add)
                else:
                    vector.wait_ge(dma_in, 16 * (t + 1) if t < 3 else 48)
                    # w = -0.001*uu + Ip ; v = sq2 + w
                    vector.scalar_tensor_tensor(w[:], uu[:], -0.001, I_t,
                                                Alu.mult, Alu.add)
                    vector.tensor_tensor(
                        v[:], sq2[:], w[:], op=Alu.add)._wait_ge(act_sem, t)
                # clip to [-90, 40]; signals ACT to start sq2(t+1)
                vector.tensor_scalar(
                    v[:], v[:], 40.0, -90.0, Alu.min, Alu.max
                ).then_inc(dve2act, 1)
                # spike mask; accum_out gives the per-partition spike count
                vector.tensor_scalar(msk[:], v[:], 30.0, None, Alu.is_ge,
                                     Alu.add, accum_out=cnt[:, t:t + 1])
                # uu' = 0.99*uu + v'   (uu0 = 500*(-13) = -6500)
                if t == 0:
                    vector.tensor_scalar(uu[:], v[:], -6435.0, None, Alu.add)
                else:
                    vector.scalar_tensor_tensor(uu[:], uu[:], 0.99, v[:],
                                                Alu.mult, Alu.add)

            # pack outputs: CA1 column broadcast + the 4 count columns
            vector.tensor_copy(
                outt[:, :COLS], v[:, COLS + 2:COLS + 3].to_broadcast((P, COLS)))
            vector.tensor_copy(outt[:, COLS:], cnt[:]).then_inc(dve_done, 1)

    return nc


def _get_program():
    global _BUILT
    if _BUILT is None:
        _BUILT = _build_program()
    return _BUILT


def _host_uniform_chain(max_abs_drive):
    """Replicates the uniform DG/CA3/CA1 dynamics in f32 on host.

    Returns (c1_v_scalar, clean) where clean additionally certifies that no
    uniform population or inhibitory LIF unit could have crossed threshold.
    """
    f = np.float32
    v = np.full(3, -65.0, f)
    u = np.full(3, -13.0, f)
    clean = True
    for _ in range(T):
        vp = np.clip(v + (f(0.04) * v * v + f(5.0) * v + f(140.0) - u) * f(DT),
                     -90.0, 40.0).astype(f)
        u = (u + f(A) * (f(B) * vp - u) * f(DT)).astype(f)
        if np.any(vp >= 29.0):  # margin below the 30.0 threshold
            clean = False
        v = vp
    # inhibitory LIF with zero input stays at 0 < THR_I; nothing to check.
    return float(v[2]), clean


def _reference_fallback(inputs):
    """Bit-faithful host replication of the reference model (slow path)."""
    f = np.float32
    d = inputs

    def transmit(spk, src, tgt, val, n_tgt):
        w = (val * spk[src]).astype(f)
        return np.bincount(tgt, weights=w, minlength=n_tgt).astype(f)

    def izh(v, u, c, dd, I):
        v = np.clip(v + (f(0.04) * v * v + f(5.0) * v + f(140.0) - u + I) * f(DT),
                    -90.0, 40.0).astype(f)
        u = (u + f(A) * (f(B) * v - u) * f(DT)).astype(f)
        s = (v >= 30.0).astype(f)
        return np.where(s > 0, c, v).astype(f), np.where(s > 0, u + dd, u).astype(f), s

    def lif(v, inp):
        v = (f(TAU_I) * v + f(1.0 - TAU_I) * inp).astype(f)
        s = (v >= THR_I).astype(f)
        return np.where(s > 0, 0.0, v).astype(f), s

    ec_v = np.full(N_EC, -65.0, f); ec_u = np.full(N_EC, B * -65.0, f)
    dg_v = np.full(N_DG, -65.0, f); dg_u = np.full(N_DG, B * -65.0, f)
    c3_v = np.full(N_CA3, -65.0, f); c3_u = np.full(N_CA3, B * -65.0, f)
    c1_v = np.full(N_CA1, -65.0, f); c1_u = np.full(N_CA1, B * -65.0, f)
    c3_s = np.zeros(N_CA3, f)
    iv_dg = np.zeros(N_I_DG, f); iv_c3 = np.zeros(N_I_CA3, f); iv_c1 = np.zeros(N_I_CA1, f)

    for t in range(T):
        I_ec = d["drive"][t]
        ec_v, ec_u, ec_s = izh(ec_v, ec_u, d["ec_c"], d["ec_d"], I_ec)
        dg_I = transmit(ec_s, d["pp_src"], d["pp_tgt"], d["pp_val"], N_DG)
        iv_dg, is_dg = lif(iv_dg, np.full(N_I_DG, dg_I.mean(), f))
        dg_v, dg_u, dg_s = izh(dg_v, dg_u, d["dg_c"], d["dg_d"],
                               dg_I - f(INH_GAIN) * is_dg.mean(dtype=f))
        c3_I = (transmit(dg_s, d["mf_src"], d["mf_tgt"], d["mf_val"], N_CA3)
                + transmit(c3_s, d["rc_src"], d["rc_tgt"], d["rc_val"], N_CA3))
        iv_c3, is_c3 = lif(iv_c3, np.full(N_I_CA3, c3_I.mean(), f))
        c3_v, c3_u, c3_s = izh(c3_v, c3_u, d["ca3_c"], d["ca3_d"],
                               c3_I - f(INH_GAIN) * is_c3.mean(dtype=f))
        c1_I = transmit(c3_s, d["sc_src"], d["sc_tgt"], d["sc_val"], N_CA1)
        iv_c1, is_c1 = lif(iv_c1, np.full(N_I_CA1, c1_I.mean(), f))
        c1_v, c1_u, c1_s = izh(c1_v, c1_u, d["ca1_c"], d["ca1_d"],
                               c1_I - f(INH_GAIN) * is_c1.mean(dtype=f))
    return c1_v


def make_in_maps(drive):
    """Per-core input maps: pre-scale drive to 0.5*I + 70 (the constant part
    of the izh update) and pad EC with silent neurons (I=0 -> prescaled 70,
    same init state -> never spike).  The 3 chain columns also carry I=0,
    i.e. 70 after prescale."""
    drive = np.asarray(drive, dtype=np.float32)
    drive_pre = (np.float32(0.5) * drive + np.float32(_IP0)).astype(np.float32)
    drive_pad = np.full((T, EC_PAD), _IP0, np.float32)
    drive_pad[:, :N_EC] = drive_pre

    in_maps = []
    for k in range(N_CORES):
        shard = drive_pad[:, k * EC_CORE:(k + 1) * EC_CORE]      # [T, 12544]
        shard = shard.reshape(T, P, COLS).transpose(1, 0, 2)      # [P, T, COLS]
        arr = np.full((P, T, COLS_ALL), _IP0, np.float32)
        arr[:, :, :COLS] = shard
        in_maps.append({"drive": np.ascontiguousarray(arr).reshape(P, T * COLS_ALL)})
    return in_maps


def kernel(**inputs):
    from concourse.bass_utils import run_bass_kernel_spmd

    drive = np.asarray(inputs["drive"], dtype=np.float32)
    assert drive.shape == (T, N_EC)
    in_maps = make_in_maps(drive)
    nc = _get_program()
    res = run_bass_kernel_spmd(nc, in_maps, list(range(N_CORES)))

    counts = np.zeros(T, np.float64)
    c1_vals = []
    for k in range(N_CORES):
        out = np.asarray(res.results[k]["out_all"], np.float32).reshape(P, COLS + T)
        counts += out[:, COLS:].astype(np.float64).sum(axis=0)
        c1_vals.append(out[:, :COLS].reshape(-1))
    c1_vals = np.concatenate(c1_vals)  # uniform CA1 value, replicated per lane

    c1_scalar, chain_clean = _host_uniform_chain(float(np.abs(drive).max()))
    if counts.sum() == 0 and chain_clean and np.all(c1_vals == c1_vals[0]):
        # cross-check device uniform value against the host chain
        if abs(float(c1_vals[0]) - c1_scalar) > 1e-3:
            return _reference_fallback(inputs)
        return np.full(N_CA1, c1_vals[0], np.float32)
    # spikes occurred: exact (slow) host fallback
    return _reference_fallback(inputs)


# revision 26
# speedup vs baseline: 1.1243x; 1.0420x over previous
"""Trainium2 Bass kernel for nn_DynamicHippocampus (spiking hippocampus network).

Network: EC --pp--> DG --mf--> CA3 (--rc--> CA3) --sc--> CA1, T=4 Izhikevich
steps, output = final CA1 membrane potential.

Strategy
--------
The only data-dependent, non-uniform computation in this network is the EC
population (per-neuron drive).  DG/CA3/CA1 all start from identical state
(v=-65, u=-13) and receive spatially-uniform input for as long as no source
population has spiked (their synaptic currents are exactly zero, and the
inhibitory LIF populations provably stay at zero as well).  So while no spikes
have occurred, DG/CA3/CA1 evolve as uniform "scalar" populations.

The device kernel (SPMD over 8 NeuronCores, EC sharded by neuron index)
computes:
  * the full per-neuron EC Izhikevich dynamics for its EC shard, and the
    per-step EC spike counts (the certificate that the no-spike regime holds),
  * the uniform DG/CA3/CA1 scalar chains (replicated, ~3 lanes of work),
  * its shard of the CA1 output (broadcast of the uniform CA1 potential).

The host verifies the certificate (device-reported EC spike counts are zero
for every step, and the uniform chains never cross threshold).  If the
certificate holds -- it does for any drive bounded well below ~190, and the
model's drive is < 20 -- the device output is exact.  Otherwise kernel()
falls back to a bit-exact reference simulation on host (slow path; never
taken for in-distribution inputs, kept for correctness on arbitrary ones).
"""

import numpy as np

# population sizes (must match the model)
N_EC, N_DG, N_CA3, N_CA1 = 100000, 400000, 120000, 100000
N_I_DG, N_I_CA3, N_I_CA1 = 10000, 3000, 2000
T, DT = 4, 0.5
A, B = 0.02, 0.2
TAU_I, THR_I, INH_GAIN = 0.9, 1.0, 2.0

# ACT computes sq2 = Square(a*v + b) = 0.02 v^2 + 3.5 v + b^2, so the DVE
# update is v' = sq2 + (0.5 I - (b^2 - 70)) - 0.001*uu.
_SQ_A = float(np.float32(np.sqrt(np.float32(0.02))))
_SQ_B = float(np.float32(3.5 / (2.0 * _SQ_A)))
_SQ_B2 = float(np.float32(_SQ_B) * np.float32(_SQ_B))
_IP0 = float(np.float32(70.0 - _SQ_B2))          # prescale constant for I=0

N_CORES = 8
P = 128          # SBUF partitions
COLS = 98        # free-dim columns of EC neurons per core
CH = 3           # extra columns carrying the uniform DG/CA3/CA1 chains
COLS_ALL = COLS + CH
EC_CORE = P * COLS            # 12544 EC neurons per core
EC_PAD = N_CORES * EC_CORE    # 100352 >= N_EC

_BUILT = None


def _build_program():
    """Build the (per-core identical) Bass program once.

    Single-engine (DVE) izh update over a [128, 101] state tile: 98 columns of
    EC neurons plus 3 columns carrying the uniform DG/CA3/CA1 chains (their
    drive columns are zero).  The host pre-scales drive to 0.5*I + 70 and the
    recovery variable is kept as uu = 500*u, so a step is ~7 fused DVE ops:
        v'  = clip(v*v*0.02 + 3.5*v + (0.5*I + 70) - 0.001*uu, -90, 40)
        uu' = 0.99*uu + v'
    (step 0 is closed-form since v,u start uniform).  Spike counts come from
    the compare op's accum_out; DMA-in is split so step-0 compute overlaps
    the bulk of the drive transfer.
    """
    import contextlib

    import concourse.bass as bass
    import concourse.mybir as mybir

    f32 = mybir.dt.float32
    Alu = mybir.AluOpType
    X = mybir.AxisListType.X

    # The kernel's cross-engine dependencies are fully semaphore-protected
    # (DMA-in -> DVE -> DMA-out), so the framework's all-engine barriers
    # (const-AP init, block entry/exit) only serialize engine boot; skip them.
    class _NoBarrierBass(bass.Bass):
        def all_engine_barrier(self, *, sem_only: bool = False):
            pass

    nc = _NoBarrierBass(detect_race_conditions=False)
    drive_d = nc.declare_dram_parameter("drive", [P, T * COLS_ALL], f32, isOutput=False)
    out_d = nc.declare_dram_parameter("out_all", [P, COLS + T], f32, isOutput=True)

    stk = contextlib.ExitStack()
    with stk:
        drv = stk.enter_context(nc.sbuf_tensor([P, T * COLS_ALL], f32))
        v = stk.enter_context(nc.sbuf_tensor([P, COLS_ALL], f32))
        uu = stk.enter_context(nc.sbuf_tensor([P, COLS_ALL], f32))
        cnt = stk.enter_context(nc.sbuf_tensor([P, T], f32))
        sq2 = stk.enter_context(nc.sbuf_tensor([P, COLS_ALL], f32))
        w = stk.enter_context(nc.sbuf_tensor([P, COLS_ALL], f32))
        msk = stk.enter_context(nc.sbuf_tensor([P, COLS_ALL], f32))
        outt = stk.enter_context(nc.sbuf_tensor([P, COLS + T], f32))
        act_a = stk.enter_context(nc.sbuf_tensor([P, 1], f32))
        act_b = stk.enter_context(nc.sbuf_tensor([P, 1], f32))
        dma_in = stk.enter_context(nc.semaphore("dma_in"))
        dve2act = stk.enter_context(nc.semaphore("dve2act"))
        act_sem = stk.enter_context(nc.semaphore("act_sem"))
        dve_done = stk.enter_context(nc.semaphore("dve_done"))
        dma_out = stk.enter_context(nc.semaphore("dma_out"))
        block = stk.enter_context(nc.Block(no_gpsimd_drain=True))

        Act = mybir.ActivationFunctionType

        @block.sync
        def _(sync):
            # 3-way split: step 0 / step 1 / steps 2-3; one queue completes
            # in issue order, so a single counting semaphore is race-free
            sync.dma_start(drv[:, :COLS_ALL],
                           drive_d[:, :COLS_ALL]).then_inc(dma_in, 16)
            sync.dma_start(drv[:, COLS_ALL:2 * COLS_ALL],
                           drive_d[:, COLS_ALL:2 * COLS_ALL]).then_inc(dma_in, 16)
            sync.dma_start(drv[:, 2 * COLS_ALL:],
                           drive_d[:, 2 * COLS_ALL:]).then_inc(dma_in, 16)
            sync.wait_ge(dve_done, 1)
            # no explicit completion wait: the NRT end-of-stream DRAIN on the
            # sync engine drains its DGE queues before the program retires
            sync.dma_start(out_d[:], outt[:]).then_inc(dma_out, 16)

        @block.scalar
        def _(scalar):
            # warm the Square LUT during the DMA window, then produce
            # sq2(t) = Square(a*v(t-1) + b) as soon as clip(t-1) lands.
            scalar.wait_ge(dve2act, 1)   # act_a/act_b written
            scalar.activation(sq2[:, 0:1], act_a[:], Act.Square,
                              bias=act_b[:], scale=act_a[:])
            for t in range(1, T):
                scalar.activation(
                    sq2[:], v[:], Act.Square, bias=act_b[:], scale=act_a[:]
                )._wait_ge(dve2act, t + 1).then_inc(act_sem, 1)

        @block.vector
        def _(vector):
            # DVE drains its 8-stage pipe after every op, so same-engine RAW
            # needs no semaphores; only DMA/ACT boundaries are synced.
            #
            # State:  v = membrane potential, uu = 500*u (scaled recovery so
            # its update fuses into one op: uu' = 0.99*uu + v').
            # Update: v' = clip(sq2 + Ip - 0.001*uu, -90, 40) with
            #         sq2 = 0.02 v^2 + 3.5 v + b^2 (ACT), Ip = 0.5*I + 70 - b^2.
            # Step 0 is closed-form: v=-65, u=-13 everywhere, so
            # v'0 = Ip + 16.625 and uu'0 = v'0 - 6435.
            vector.memset(act_a[:], _SQ_A)
            vector.memset(act_b[:], _SQ_B).then_inc(dve2act, 1)
            vector.wait_ge(dma_in, 16)
            for t in range(T):
                I_t = drv[:, t * COLS_ALL:(t + 1) * COLS_ALL]
                if t == 0:
                    vector.tensor_scalar(v[:], I_t, 16.625, None, Alu.add)
                else:
                    vector.wait_ge(dma_in, 16 * (t + 1) if t < 3 else 48)
                    # w = -0.001*uu + Ip ; v = sq2 + w
                    vector.scalar_tensor_tensor(w[:], uu[:], -0.001, I_t,
                                                Alu.mult, Alu.add)
                    vector.tensor_tensor(
                        v[:], sq2[:], w[:], op=Alu.add)._wait_ge(act_sem, t)
                # clip to [-90, 40]; signals ACT to start sq2(t+1)
                vector.tensor_scalar(
                    v[:], v[:], 40.0, -90.0, Alu.min, Alu.max
                ).then_inc(dve2act, 1)
                # spike mask; accum_out gives the per-partition spike count
                vector.tensor_scalar(msk[:], v[:], 30.0, None, Alu.is_ge,
                                     Alu.add, accum_out=cnt[:, t:t + 1])
                # uu' = 0.99*uu + v'   (uu0 = 500*(-13) = -6500)
                if t == 0:
                    vector.tensor_scalar(uu[:], v[:], -6435.0, None, Alu.add)
                else:
                    vector.scalar_tensor_tensor(uu[:], uu[:], 0.99, v[:],
                                                Alu.mult, Alu.add)

            # pack outputs: CA1 column broadcast + the 4 count columns
            vector.tensor_copy(
                outt[:, :COLS], v[:, COLS + 2:COLS + 3].to_broadcast((P, COLS)))
            vector.tensor_copy(outt[:, COLS:], cnt[:]).then_inc(dve_done, 1)

    return nc


def _get_program():
    global _BUILT
    if _BUILT is None:
        _BUILT = _build_program()
    return _BUILT


def _host_uniform_chain(max_abs_drive):
    """Replicates the uniform DG/CA3/CA1 dynamics in f32 on host.

    Returns (c1_v_scalar, clean) where clean additionally certifies that no
    uniform population or inhibitory LIF unit could have crossed threshold.
    """
    f = np.float32
    v = np.full(3, -65.0, f)
    u = np.full(3, -13.0, f)
    clean = True
    for _ in range(T):
        vp = np.clip(v + (f(0.04) * v * v + f(5.0) * v + f(140.0) - u) * f(DT),
                     -90.0, 40.0).astype(f)
        u = (u + f(A) * (f(B) * vp - u) * f(DT)).astype(f)
        if np.any(vp >= 29.0):  # margin below the 30.0 threshold
            clean = False
        v = vp
    # inhibitory LIF with zero input stays at 0 < THR_I; nothing to check.
    return float(v[2]), clean


def _reference_fallback(inputs):
    """Bit-faithful host replication of the reference model (slow path)."""
    f = np.float32
    d = inputs

    def transmit(spk, src, tgt, val, n_tgt):
        w = (val * spk[src]).astype(f)
        return np.bincount(tgt, weights=w, minlength=n_tgt).astype(f)

    def izh(v, u, c, dd, I):
        v = np.clip(v + (f(0.04) * v * v + f(5.0) * v + f(140.0) - u + I) * f(DT),
                    -90.0, 40.0).astype(f)
        u = (u + f(A) * (f(B) * v - u) * f(DT)).astype(f)
        s = (v >= 30.0).astype(f)
        return np.where(s > 0, c, v).astype(f), np.where(s > 0, u + dd, u).astype(f), s

    def lif(v, inp):
        v = (f(TAU_I) * v + f(1.0 - TAU_I) * inp).astype(f)
        s = (v >= THR_I).astype(f)
        return np.where(s > 0, 0.0, v).astype(f), s

    ec_v = np.full(N_EC, -65.0, f); ec_u = np.full(N_EC, B * -65.0, f)
    dg_v = np.full(N_DG, -65.0, f); dg_u = np.full(N_DG, B * -65.0, f)
    c3_v = np.full(N_CA3, -65.0, f); c3_u = np.full(N_CA3, B * -65.0, f)
    c1_v = np.full(N_CA1, -65.0, f); c1_u = np.full(N_CA1, B * -65.0, f)
    c3_s = np.zeros(N_CA3, f)
    iv_dg = np.zeros(N_I_DG, f); iv_c3 = np.zeros(N_I_CA3, f); iv_c1 = np.zeros(N_I_CA1, f)

    for t in range(T):
        I_ec = d["drive"][t]
        ec_v, ec_u, ec_s = izh(ec_v, ec_u, d["ec_c"], d["ec_d"], I_ec)
        dg_I = transmit(ec_s, d["pp_src"], d["pp_tgt"], d["pp_val"], N_DG)
        iv_dg, is_dg = lif(iv_dg, np.full(N_I_DG, dg_I.mean(), f))
        dg_v, dg_u, dg_s = izh(dg_v, dg_u, d["dg_c"], d["dg_d"],
                               dg_I - f(INH_GAIN) * is_dg.mean(dtype=f))
        c3_I = (transmit(dg_s, d["mf_src"], d["mf_tgt"], d["mf_val"], N_CA3)
                + transmit(c3_s, d["rc_src"], d["rc_tgt"], d["rc_val"], N_CA3))
        iv_c3, is_c3 = lif(iv_c3, np.full(N_I_CA3, c3_I.mean(), f))
        c3_v, c3_u, c3_s = izh(c3_v, c3_u, d["ca3_c"], d["ca3_d"],
                               c3_I - f(INH_GAIN) * is_c3.mean(dtype=f))
        c1_I = transmit(c3_s, d["sc_src"], d["sc_tgt"], d["sc_val"], N_CA1)
        iv_c1, is_c1 = lif(iv_c1, np.full(N_I_CA1, c1_I.mean(), f))
        c1_v, c1_u, c1_s = izh(c1_v, c1_u, d["ca1_c"], d["ca1_d"],
                               c1_I - f(INH_GAIN) * is_c1.mean(dtype=f))
    return c1_v


def make_in_maps(drive):
    """Per-core input maps: pre-scale drive to 0.5*I + 70 (the constant part
    of the izh update) and pad EC with silent neurons (I=0 -> prescaled 70,
    same init state -> never spike).  The 3 chain columns also carry I=0,
    i.e. 70 after prescale."""
    drive = np.asarray(drive, dtype=np.float32)
    drive_pre = (np.float32(0.5) * drive + np.float32(_IP0)).astype(np.float32)
    drive_pad = np.full((T, EC_PAD), _IP0, np.float32)
    drive_pad[:, :N_EC] = drive_pre

    in_maps = []
    for k in range(N_CORES):
        shard = drive_pad[:, k * EC_CORE:(k + 1) * EC_CORE]      # [T, 12544]
        shard = shard.reshape(T, P, COLS).transpose(1, 0, 2)      # [P, T, COLS]
        arr = np.full((P, T, COLS_ALL), _IP0, np.float32)
        arr[:, :, :COLS] = shard
        in_maps.append({"drive": np.ascontiguousarray(arr).reshape(P, T * COLS_ALL)})
    return in_maps


def kernel(**inputs):
    from concourse.bass_utils import run_bass_kernel_spmd

    drive = np.asarray(inputs["drive"], dtype=np.float32)
    assert drive.shape == (T, N_EC)
    in_maps = make_in_maps(drive)
    nc = _get_program()
    res = run_bass_kernel_spmd(nc, in_maps, list(range(N_CORES)))

    counts = np.zeros(T, np.float64)
    c1_vals = []
    for k in range(N_CORES):
        out = np.asarray(res.results[k]["out_all"], np.float32).reshape(P, COLS + T)
        counts += out[:, COLS:].astype(np.float64).sum(axis=0)
        c1_vals.append(out[:, :COLS].reshape(-1))
    c1_vals = np.concatenate(c1_vals)  # uniform CA1 value, replicated per lane

    c1_scalar, chain_clean = _host_uniform_chain(float(np.abs(drive).max()))
    if counts.sum() == 0 and chain_clean and np.all(c1_vals == c1_vals[0]):
        # cross-check device uniform value against the host chain
        if abs(float(c1_vals[0]) - c1_scalar) > 1e-3:
            return _reference_fallback(inputs)
        return np.full(N_CA1, c1_vals[0], np.float32)
    # spikes occurred: exact (slow) host fallback
    return _reference_fallback(inputs)


# revision 29
# speedup vs baseline: 1.2723x; 1.1316x over previous
"""Trainium2 Bass kernel for nn_DynamicHippocampus (spiking hippocampus network).

Network: EC --pp--> DG --mf--> CA3 (--rc--> CA3) --sc--> CA1, T=4 Izhikevich
steps, output = final CA1 membrane potential.

Strategy
--------
The only data-dependent, non-uniform computation in this network is the EC
population (per-neuron drive).  DG/CA3/CA1 all start from identical state
(v=-65, u=-13) and receive spatially-uniform input for as long as no source
population has spiked (their synaptic currents are exactly zero, and the
inhibitory LIF populations provably stay at zero as well).  So while no spikes
have occurred, DG/CA3/CA1 evolve as uniform "scalar" populations.

The device kernel (SPMD over 8 NeuronCores, EC sharded by neuron index)
computes:
  * the full per-neuron EC Izhikevich dynamics for its EC shard, and the
    per-step EC spike counts (the certificate that the no-spike regime holds),
  * the uniform DG/CA3/CA1 scalar chains (replicated, ~3 lanes of work),
  * its shard of the CA1 output (broadcast of the uniform CA1 potential).

The host verifies the certificate (device-reported EC spike counts are zero
for every step, and the uniform chains never cross threshold).  If the
certificate holds -- it does for any drive bounded well below ~190, and the
model's drive is < 20 -- the device output is exact.  Otherwise kernel()
falls back to a bit-exact reference simulation on host (slow path; never
taken for in-distribution inputs, kept for correctness on arbitrary ones).
"""

import numpy as np

# population sizes (must match the model)
N_EC, N_DG, N_CA3, N_CA1 = 100000, 400000, 120000, 100000
N_I_DG, N_I_CA3, N_I_CA1 = 10000, 3000, 2000
T, DT = 4, 0.5
A, B = 0.02, 0.2
TAU_I, THR_I, INH_GAIN = 0.9, 1.0, 2.0

# ACT computes sq2 = Square(a*v + b) = 0.02 v^2 + 3.5 v + b^2, so the DVE
# update is v' = sq2 + (0.5 I - (b^2 - 70)) - 0.001*uu.
_SQ_A = float(np.float32(np.sqrt(np.float32(0.02))))
_SQ_B = float(np.float32(3.5 / (2.0 * _SQ_A)))
_SQ_B2 = float(np.float32(_SQ_B) * np.float32(_SQ_B))
_IP0 = float(np.float32(70.0 - _SQ_B2))          # prescale constant for I=0

N_CORES = 8
P = 128          # SBUF partitions
COLS = 98        # free-dim columns of EC neurons per core
CH = 3           # extra columns carrying the uniform DG/CA3/CA1 chains
COLS_ALL = COLS + CH
EC_CORE = P * COLS            # 12544 EC neurons per core
EC_PAD = N_CORES * EC_CORE    # 100352 >= N_EC

_BUILT = None


def _build_program():
    """Build the (per-core identical) Bass program once.

    Izhikevich update over a [128, 101] state tile: 98 columns of EC neurons
    plus 3 columns carrying the uniform DG/CA3/CA1 chains (their drive
    columns are zero).  The Scalar engine (ACT) computes the quadratic
    sq2 = Square(a*v + b) = 0.02 v^2 + 3.5 v + b^2 pipelined one step ahead
    of the Vector engine (DVE), which does 5 fused ops per step:
        v'  = clip(sq2 + (0.5*I + 70 - b^2) - 0.001*uu, -90, 40)
        uu' = 0.99*uu + v'        (uu = 500*u keeps this a single op)
    Step 0 is closed-form (v,u start uniform); the host pre-scales drive by
    0.5*I + 70 - b^2; spike counts come from the compare op's accum_out; the
    drive DMA is split in three so compute overlaps the transfer; the ACT
    Square-LUT load is hidden in the initial DMA window.
    """
    import contextlib

    import concourse.bass as bass
    import concourse.mybir as mybir

    f32 = mybir.dt.float32
    Alu = mybir.AluOpType
    X = mybir.AxisListType.X

    # The kernel's cross-engine dependencies are fully semaphore-protected
    # (DMA-in -> DVE -> DMA-out), so the framework's all-engine barriers
    # (const-AP init, block entry/exit) only serialize engine boot; skip them.
    class _NoBarrierBass(bass.Bass):
        def all_engine_barrier(self, *, sem_only: bool = False):
            pass

    nc = _NoBarrierBass(detect_race_conditions=False)
    drive_d = nc.declare_dram_parameter("drive", [P, T * COLS_ALL], f32, isOutput=False)
    out_d = nc.declare_dram_parameter("out_all", [P, COLS + T], f32, isOutput=True)

    stk = contextlib.ExitStack()
    with stk:
        drv = stk.enter_context(nc.sbuf_tensor([P, T * COLS_ALL], f32))
        v = stk.enter_context(nc.sbuf_tensor([P, COLS_ALL], f32))
        uu = stk.enter_context(nc.sbuf_tensor([P, COLS_ALL], f32))
        cnt = stk.enter_context(nc.sbuf_tensor([P, T], f32))
        sq2 = stk.enter_context(nc.sbuf_tensor([P, COLS_ALL], f32))
        w = stk.enter_context(nc.sbuf_tensor([P, COLS_ALL], f32))
        outt = stk.enter_context(nc.sbuf_tensor([P, COLS + T], f32))
        act_a = stk.enter_context(nc.sbuf_tensor([P, 1], f32))
        act_b = stk.enter_context(nc.sbuf_tensor([P, 1], f32))
        dma_in = stk.enter_context(nc.semaphore("dma_in"))
        dma_in_b = stk.enter_context(nc.semaphore("dma_in_b"))
        dve2act = stk.enter_context(nc.semaphore("dve2act"))
        act_sem = stk.enter_context(nc.semaphore("act_sem"))
        dve_done = stk.enter_context(nc.semaphore("dve_done"))
        dma_out = stk.enter_context(nc.semaphore("dma_out"))
        block = stk.enter_context(nc.Block(no_gpsimd_drain=True))

        Act = mybir.ActivationFunctionType

        @block.sync
        def _(sync):
            # 3-way split (step 0 / step 1 / steps 2-3); step 0 is further
            # halved across the sync and scalar DMA queues.  Per-queue
            # completions are in issue order, so one counting semaphore per
            # queue is race-free.
            sync.dma_start(drv[:, :52],
                           drive_d[:, :52]).then_inc(dma_in, 16)
            sync.dma_start(drv[:, COLS_ALL:2 * COLS_ALL],
                           drive_d[:, COLS_ALL:2 * COLS_ALL]).then_inc(dma_in, 16)
            sync.dma_start(drv[:, 2 * COLS_ALL:],
                           drive_d[:, 2 * COLS_ALL:]).then_inc(dma_in, 16)
            sync.wait_ge(dve_done, 1)
            # no explicit completion wait: the NRT end-of-stream DRAIN on the
            # sync engine drains its DGE queues before the program retires
            sync.dma_start(out_d[:], outt[:]).then_inc(dma_out, 16)

        @block.scalar
        def _(scalar):
            scalar.dma_start(drv[:, 52:COLS_ALL],
                             drive_d[:, 52:COLS_ALL]).then_inc(dma_in_b, 16)
            # warm the Square LUT during the DMA window, then produce
            # sq2(t) = Square(a*v(t-1) + b) as soon as clip(t-1) lands.
            scalar.wait_ge(dve2act, 1)   # act_a/act_b written
            scalar.activation(sq2[:, 0:1], act_a[:], Act.Square,
                              bias=act_b[:], scale=act_a[:])
            for t in range(1, T):
                scalar.activation(
                    sq2[:], v[:], Act.Square, bias=act_b[:], scale=act_a[:]
                )._wait_ge(dve2act, t + 1).then_inc(act_sem, 1)

        @block.vector
        def _(vector):
            # DVE drains its 8-stage pipe after every op, so same-engine RAW
            # needs no semaphores; only DMA/ACT boundaries are synced.
            #
            # State:  v = membrane potential, uu = 500*u (scaled recovery so
            # its update fuses into one op: uu' = 0.99*uu + v').
            # Update: v' = clip(sq2 + Ip - 0.001*uu, -90, 40) with
            #         sq2 = 0.02 v^2 + 3.5 v + b^2 (ACT), Ip = 0.5*I + 70 - b^2.
            # Step 0 is closed-form: v=-65, u=-13 everywhere, so
            # v'0 = Ip + 16.625 and uu'0 = v'0 - 6435.
            vector.memset(act_a[:], _SQ_A)
            vector.memset(act_b[:], _SQ_B).then_inc(dve2act, 1)
            vector.wait_ge(dma_in, 16)
            vector.wait_ge(dma_in_b, 16)
            for t in range(T):
                I_t = drv[:, t * COLS_ALL:(t + 1) * COLS_ALL]
                if t == 0:
                    # ACT reads the UNCLIPPED v: kernel() verifies drive >= 0,
                    # under which v' >= -87.2 (the -90 clip never engages) and
                    # any v' >= 30 voids the certificate anyway.
                    vector.tensor_scalar(
                        v[:], I_t, 16.625, None, Alu.add).then_inc(dve2act, 1)
                else:
                    vector.wait_ge(dma_in, 16 * (t + 1) if t < 3 else 48)
                    # w = -0.001*uu + Ip ; v = sq2 + w
                    vector.scalar_tensor_tensor(w[:], uu[:], -0.001, I_t,
                                                Alu.mult, Alu.add)
                    vector.tensor_tensor(
                        v[:], sq2[:], w[:], op=Alu.add
                    )._wait_ge(act_sem, t).then_inc(dve2act, 1)
                # clip to [-90, 40]; accum_out (op1=max) records the
                # per-partition max -- the spike certificate is max < 30.
                vector.tensor_scalar(
                    v[:], v[:], 40.0, -90.0, Alu.min, Alu.max,
                    accum_out=cnt[:, t:t + 1])
                # uu' = 0.99*uu + v'   (uu0 = 500*(-13) = -6500)
                if t == 0:
                    vector.tensor_scalar(uu[:], v[:], -6435.0, None, Alu.add)
                else:
                    vector.scalar_tensor_tensor(uu[:], uu[:], 0.99, v[:],
                                                Alu.mult, Alu.add)

            # pack outputs: CA1 column broadcast + the 4 count columns
            vector.tensor_copy(
                outt[:, :COLS], v[:, COLS + 2:COLS + 3].to_broadcast((P, COLS)))
            vector.tensor_copy(outt[:, COLS:], cnt[:]).then_inc(dve_done, 1)

    return nc


def _get_program():
    global _BUILT
    if _BUILT is None:
        _BUILT = _build_program()
    return _BUILT


def _host_uniform_chain(max_abs_drive):
    """Replicates the uniform DG/CA3/CA1 dynamics in f32 on host.

    Returns (c1_v_scalar, clean) where clean additionally certifies that no
    uniform population or inhibitory LIF unit could have crossed threshold.
    """
    f = np.float32
    v = np.full(3, -65.0, f)
    u = np.full(3, -13.0, f)
    clean = True
    for _ in range(T):
        vp = np.clip(v + (f(0.04) * v * v + f(5.0) * v + f(140.0) - u) * f(DT),
                     -90.0, 40.0).astype(f)
        u = (u + f(A) * (f(B) * vp - u) * f(DT)).astype(f)
        if np.any(vp >= 29.0):  # margin below the 30.0 threshold
            clean = False
        v = vp
    # inhibitory LIF with zero input stays at 0 < THR_I; nothing to check.
    return float(v[2]), clean


def _reference_fallback(inputs):
    """Bit-faithful host replication of the reference model (slow path)."""
    f = np.float32
    d = inputs

    def transmit(spk, src, tgt, val, n_tgt):
        w = (val * spk[src]).astype(f)
        return np.bincount(tgt, weights=w, minlength=n_tgt).astype(f)

    def izh(v, u, c, dd, I):
        v = np.clip(v + (f(0.04) * v * v + f(5.0) * v + f(140.0) - u + I) * f(DT),
                    -90.0, 40.0).astype(f)
        u = (u + f(A) * (f(B) * v - u) * f(DT)).astype(f)
        s = (v >= 30.0).astype(f)
        return np.where(s > 0, c, v).astype(f), np.where(s > 0, u + dd, u).astype(f), s

    def lif(v, inp):
        v = (f(TAU_I) * v + f(1.0 - TAU_I) * inp).astype(f)
        s = (v >= THR_I).astype(f)
        return np.where(s > 0, 0.0, v).astype(f), s

    ec_v = np.full(N_EC, -65.0, f); ec_u = np.full(N_EC, B * -65.0, f)
    dg_v = np.full(N_DG, -65.0, f); dg_u = np.full(N_DG, B * -65.0, f)
    c3_v = np.full(N_CA3, -65.0, f); c3_u = np.full(N_CA3, B * -65.0, f)
    c1_v = np.full(N_CA1, -65.0, f); c1_u = np.full(N_CA1, B * -65.0, f)
    c3_s = np.zeros(N_CA3, f)
    iv_dg = np.zeros(N_I_DG, f); iv_c3 = np.zeros(N_I_CA3, f); iv_c1 = np.zeros(N_I_CA1, f)

    for t in range(T):
        I_ec = d["drive"][t]
        ec_v, ec_u, ec_s = izh(ec_v, ec_u, d["ec_c"], d["ec_d"], I_ec)
        dg_I = transmit(ec_s, d["pp_src"], d["pp_tgt"], d["pp_val"], N_DG)
        iv_dg, is_dg = lif(iv_dg, np.full(N_I_DG, dg_I.mean(), f))
        dg_v, dg_u, dg_s = izh(dg_v, dg_u, d["dg_c"], d["dg_d"],
                               dg_I - f(INH_GAIN) * is_dg.mean(dtype=f))
        c3_I = (transmit(dg_s, d["mf_src"], d["mf_tgt"], d["mf_val"], N_CA3)
                + transmit(c3_s, d["rc_src"], d["rc_tgt"], d["rc_val"], N_CA3))
        iv_c3, is_c3 = lif(iv_c3, np.full(N_I_CA3, c3_I.mean(), f))
        c3_v, c3_u, c3_s = izh(c3_v, c3_u, d["ca3_c"], d["ca3_d"],
                               c3_I - f(INH_GAIN) * is_c3.mean(dtype=f))
        c1_I = transmit(c3_s, d["sc_src"], d["sc_tgt"], d["sc_val"], N_CA1)
        iv_c1, is_c1 = lif(iv_c1, np.full(N_I_CA1, c1_I.mean(), f))
        c1_v, c1_u, c1_s = izh(c1_v, c1_u, d["ca1_c"], d["ca1_d"],
                               c1_I - f(INH_GAIN) * is_c1.mean(dtype=f))
    return c1_v


def make_in_maps(drive):
    """Per-core input maps: pre-scale drive to 0.5*I + 70 (the constant part
    of the izh update) and pad EC with silent neurons (I=0 -> prescaled 70,
    same init state -> never spike).  The 3 chain columns also carry I=0,
    i.e. 70 after prescale."""
    drive = np.asarray(drive, dtype=np.float32)
    drive_pre = (np.float32(0.5) * drive + np.float32(_IP0)).astype(np.float32)
    drive_pad = np.full((T, EC_PAD), _IP0, np.float32)
    drive_pad[:, :N_EC] = drive_pre

    in_maps = []
    for k in range(N_CORES):
        shard = drive_pad[:, k * EC_CORE:(k + 1) * EC_CORE]      # [T, 12544]
        shard = shard.reshape(T, P, COLS).transpose(1, 0, 2)      # [P, T, COLS]
        arr = np.full((P, T, COLS_ALL), _IP0, np.float32)
        arr[:, :, :COLS] = shard
        in_maps.append({"drive": np.ascontiguousarray(arr).reshape(P, T * COLS_ALL)})
    return in_maps


def kernel(**inputs):
    from concourse.bass_utils import run_bass_kernel_spmd

    drive = np.asarray(inputs["drive"], dtype=np.float32)
    assert drive.shape == (T, N_EC)
    in_maps = make_in_maps(drive)
    nc = _get_program()
    res = run_bass_kernel_spmd(nc, in_maps, list(range(N_CORES)))

    vmax = -np.inf
    c1_vals = []
    for k in range(N_CORES):
        out = np.asarray(res.results[k]["out_all"], np.float32).reshape(P, COLS + T)
        vmax = max(vmax, float(out[:, COLS:].max()))
        c1_vals.append(out[:, :COLS].reshape(-1))
    c1_vals = np.concatenate(c1_vals)  # uniform CA1 value, replicated per lane

    c1_scalar, chain_clean = _host_uniform_chain(float(np.abs(drive).max()))
    if (vmax < 30.0 and chain_clean and np.all(c1_vals == c1_vals[0])
            and float(drive.min()) >= 0.0):
        # cross-check device uniform value against the host chain
        if abs(float(c1_vals[0]) - c1_scalar) > 1e-3:
            return _reference_fallback(inputs)
        return np.full(N_CA1, c1_vals[0], np.float32)
    # spikes occurred: exact (slow) host fallback
    return _reference_fallback(inputs)
